# revision 1
# baseline (speedup 1.0000x reference)
"""Trainium2 Bass kernel for nn_ChannelWisePatchLevelObfuscator.

Math: split each (512,512) image into 32x32 patches of 16x16; per (channel,
group) apply a dense 256->256 obfuscation matmul over patch pixels (group =
(row+col) % 32), add bias, tanh, then permute channels.

Sharding: data-parallel over batch B=64 across 8 NeuronCores (8 images/core);
weights/biases replicated (per the sharding hint). The channel permutation is
applied for free while scattering per-core results into the full output.

Layout strategy: the host packs x into a group-sorted, contraction-major
("pixel on partition") layout and pre-permutes W to match, so every device
DMA is a fully-contiguous [128 x 4KiB-per-partition] slab at peak HBM
bandwidth. A direct strided load of the patch-transposed layout would be
4-byte-granular (unusable), and on-chip PE/DVE transposes cannot express the
needed rr<->px digit swap at >=32 granularity, so the layout work belongs on
the host and the device runs at the memory roofline.

Precision: matmul inputs and the tanh output are stored as fp16 (accumulation
is fp32 in PSUM; bias+tanh on ScalarE reading fp32 PSUM). End-to-end error vs
the fp32 reference: rel ~3.6e-4, absmax ~1.6e-3 — ~7x tighter than a bf16
kernel. This halves DMA traffic (72 -> 36 MiB/core); measured HW exec time
112-128 us vs the ~106 us HBM floor for 36 MiB at 358 GB/s/core.

Device loop per core: 6 blocks of (channel, 8 groups). Per group and output
half oc, PSUM accumulates two K=128 matmuls (W chunk stationary, x streaming,
N=256); one ScalarE activation then does bias + tanh + PSUM->SBUF in fp16.
Loads issue on the SP HWDGE ring, stores on the ACT ring.
"""
import sys
import numpy as np

sys.path.insert(0, "/opt/trn_rl_repo")

import concourse.bacc as bacc  # noqa: E402
import concourse.mybir as mybir  # noqa: E402
import concourse.tile as tile  # noqa: E402
from concourse.bass_utils import run_bass_kernel_spmd  # noqa: E402

IMG, C, PS, G, B = 512, 3, 16, 32, 64
NH = NW = IMG // PS          # 32 patches per side
P2 = PS * PS                 # 256 pixels per patch
NCORES = 8
BS = B // NCORES             # 8 images per core
T = BS * NH                  # 256 matmul rows per (c, g): t = b*32 + r
GB = 8                       # groups per SBUF block (1 MiB fp16 tiles)
NGB = G // GB                # blocks per channel

F32 = mybir.dt.float32
MM_DT = mybir.dt.float16     # matmul input dtype
OUT_DT = mybir.dt.float16    # device store dtype; host upcasts to fp32
NP_MM = np.float16

_g = np.arange(G)[:, None]
_r = np.arange(NH)[None, :]
COLS = (_g - _r) % NW        # (g, r) -> patch column belonging to group g

_CACHE = {}


def _build_nc():
    nc = bacc.Bacc("TRN2", target_bir_lowering=False, debug=False,
                   num_devices=NCORES)
    # slab layouts [c, gb, 128, free]: each (c, gb) tile load/store is one
    # contiguous 4 KiB descriptor per partition.
    xt = nc.dram_tensor("xt", [C, NGB, 128, GB * 2 * T], MM_DT,
                        kind="ExternalInput")
    w = nc.dram_tensor("w", [C, NGB, 128, GB * 2 * P2], MM_DT,
                       kind="ExternalInput")
    bias = nc.dram_tensor("bias", [128, C * G * 2], F32, kind="ExternalInput")
    out = nc.dram_tensor("out", [C, NGB, 128, GB * 2 * T], OUT_DT,
                         kind="ExternalOutput")

    with tile.TileContext(nc) as tc:
        with tc.tile_pool(name="biasp", bufs=1) as bias_pool, \
             tc.tile_pool(name="xtp", bufs=5) as xt_pool, \
             tc.tile_pool(name="wp", bufs=5) as w_pool, \
             tc.tile_pool(name="outp", bufs=4) as out_pool, \
             tc.tile_pool(name="psp", bufs=8, space="PSUM") as ps_pool:
            bias_sb = bias_pool.tile([128, C * G * 2], F32)
            nc.sync.dma_start(bias_sb[:], bias[:, :])
            for c in range(C):
                for gb in range(NGB):
                    xt_t = xt_pool.tile([128, GB * 2 * T], MM_DT)
                    nc.sync.dma_start(xt_t[:], xt[c, gb])
                    w_t = w_pool.tile([128, GB * 2 * P2], MM_DT)
                    nc.sync.dma_start(w_t[:], w[c, gb])
                    out_t = out_pool.tile([128, GB * 2 * T], OUT_DT)
                    for gl in range(GB):
                        for oc in range(2):
                            ps = ps_pool.tile([128, T], F32)
                            for kc in range(2):
                                base = (gl * 2 + kc) * P2
                                nc.tensor.matmul(
                                    ps[:],
                                    w_t[:, base + oc * 128: base + oc * 128 + 128],
                                    xt_t[:, (gl * 2 + kc) * T: (gl * 2 + kc + 1) * T],
                                    start=(kc == 0), stop=(kc == 1))
                            bidx = (c * G + gb * GB + gl) * 2 + oc
                            nc.scalar.activation(
                                out_t[:, (gl * 2 + oc) * T: (gl * 2 + oc + 1) * T],
                                ps[:],
                                mybir.ActivationFunctionType.Tanh,
                                bias=bias_sb[:, bidx: bidx + 1],
                                scale=1.0)
                    nc.scalar.dma_start(out[c, gb], out_t[:])
    nc.compile()
    return nc


def _pack_xt(x_shard):
    # (BS, C, 512, 512) -> xt[c, gb, k_lo, (g_lo, kc, t)] slab layout where
    # the contraction index p=(py,px) sits on partitions (k = kc*128 + k_lo)
    xp = x_shard.reshape(BS, C, NH, PS, NW, PS)        # b c r py cl px
    sel = xp[:, :, _r, :, COLS, :]                     # g r b c py px
    xt = sel.transpose(3, 0, 4, 5, 2, 1).reshape(C, G, P2, T).astype(NP_MM)
    xt = xt.reshape(C, NGB, GB, 2, 128, T).transpose(0, 1, 4, 2, 3, 5)
    return np.ascontiguousarray(xt.reshape(C, NGB, 128, GB * 2 * T))


def _pack_w(w_full):
    # [c, g, p_in, p_out] -> [c, gb, k_lo, (g_lo, kc, o)]
    w2 = (w_full.astype(NP_MM)
          .reshape(C, NGB, GB, 2, 128, P2).transpose(0, 1, 4, 2, 3, 5))
    return np.ascontiguousarray(w2.reshape(C, NGB, 128, GB * 2 * P2))


def _unpack_out(out_dev, dst, perm):
    # out_dev[c, gb, o_lo, (g_lo, oc, t)] -> dst[b, c_final, H, W] with the
    # channel permutation folded into the scatter
    od = (out_dev.astype(np.float32)
          .reshape(C, NGB, 128, GB, 2, T).transpose(0, 1, 3, 4, 2, 5))
    o = od.reshape(C, G, P2, BS, NH)                   # c g o b r
    src = o.transpose(1, 4, 3, 0, 2).reshape(G, NH, BS, C, PS, PS)
    tmp = np.empty((NH, NW, BS, C, PS, PS), dtype=np.float32)
    tmp[_r, COLS] = src                                # tmp[r, (g-r)%32] = src[g, r]
    img = tmp.transpose(2, 3, 0, 4, 1, 5).reshape(BS, C, IMG, IMG)
    dst[:] = img[:, perm]


def kernel(x, obfuscation_weights, obfuscation_biases, channel_permutation):
    x = np.ascontiguousarray(x, dtype=np.float32)
    w = np.ascontiguousarray(obfuscation_weights, dtype=np.float32)
    bias = np.asarray(obfuscation_biases, dtype=np.float32)
    perm = np.asarray(channel_permutation, dtype=np.int64)

    if "nc" not in _CACHE:
        _CACHE["nc"] = _build_nc()
    nc = _CACHE["nc"]

    bias_t = np.ascontiguousarray(
        bias.reshape(C, G, 2, 128).transpose(3, 0, 1, 2).reshape(128, C * G * 2))
    w_packed = _pack_w(w)

    in_maps = []
    for core in range(NCORES):
        xt = _pack_xt(x[core * BS:(core + 1) * BS])
        in_maps.append({"xt": xt, "w": w_packed, "bias": bias_t})

    res = run_bass_kernel_spmd(nc, in_maps, core_ids=list(range(NCORES)))
    _CACHE["last_results"] = res

    out = np.empty((B, C, IMG, IMG), dtype=np.float32)
    for core in range(NCORES):
        _unpack_out(res.results[core]["out"],
                    out[core * BS:(core + 1) * BS], perm)
    return out



# revision 2
# speedup vs baseline: 1.3478x; 1.3478x over previous
"""Trainium2 Bass kernel for nn_ChannelWisePatchLevelObfuscator.

Math: split each (512,512) image into 32x32 patches of 16x16; per (channel,
group) apply a dense 256->256 obfuscation matmul over patch pixels (group =
(row+col) % 32), add bias, tanh, then permute channels.

Sharding: over the 96 (channel, group) pairs -- 12 pairs per core, each pair
covering the FULL batch (64 images x 32 patches = 2048 matmul rows). Unlike
batch-parallel sharding (which replicates the 12 MiB fp16 weight tensor into
every core), this loads each weight exactly once somewhere: per-core DMA
traffic drops from 36 MiB (12 x + 12 w + 12 out) to 25.5 MiB
(12 x + 1.5 w + 12 out), moving the HBM roofline from ~106 us to ~75 us at
358 GB/s/core. The channel permutation is applied for free while scattering
per-core results into the full output.

Layout strategy: the host packs x into a (pair)-sorted, contraction-major
("pixel on partition") slab layout and pre-permutes W to match, so every
device DMA is a fully contiguous >=4KiB-per-partition transfer at line rate.

Precision: matmul inputs and the tanh output are fp16 (accumulation is fp32
in PSUM; bias+tanh on ScalarE reading fp32 PSUM). Rel err vs the fp32
reference ~3.6e-4.

Device loop per core: 12 pairs; per pair and output half oc, four N=512
matmul-pairs (K=2x128) accumulate into a 4-bank [128,2048] PSUM tile, then a
single ScalarE activation does bias + tanh + PSUM->SBUF fp16 over all 2048
columns (one big ACTIVATE amortizes the ~352-cycle fixed cost that made
N=256 activations a 97us ScalarE bottleneck), and a 512 KiB store streams it
out. Loads ride the SP HWDGE ring, weights+stores the ACT ring.
"""
import sys
import numpy as np

sys.path.insert(0, "/opt/trn_rl_repo")

import concourse.bacc as bacc  # noqa: E402
import concourse.mybir as mybir  # noqa: E402
import concourse.tile as tile  # noqa: E402
from concourse.bass_utils import run_bass_kernel_spmd  # noqa: E402

IMG, C, PS, G, B = 512, 3, 16, 32, 64
NH = NW = IMG // PS          # 32 patches per side
P2 = PS * PS                 # 256 pixels per patch
NCORES = 8
NPAIR = C * G                # 96 (channel, group) pairs
PPC = NPAIR // NCORES        # 12 pairs per core
T = B * NH                   # 2048 matmul rows per pair: t = b*32 + r
NB = 4                       # N-blocks of 512 per oc half
PH = PPC // 2                # pairs per weight-load chunk

F32 = mybir.dt.float32
MM_DT = mybir.dt.float16     # matmul input dtype
OUT_DT = mybir.dt.float16    # device store dtype; host upcasts to fp32
NP_MM = np.float16

_g = np.arange(G)[:, None]
_r = np.arange(NH)[None, :]
COLS = (_g - _r) % NW        # (g, r) -> patch column belonging to group g

_CACHE = {}


def _build_nc():
    nc = bacc.Bacc("TRN2", target_bir_lowering=False, debug=False,
                   num_devices=NCORES)
    # xt[pair, k_lo, kc*2048 + t]: contraction p = kc*128 + k_lo on
    # partitions; each pair is one contiguous 1 MiB slab (8 KiB/partition).
    xt = nc.dram_tensor("xt", [PPC, 128, 2 * T], MM_DT, kind="ExternalInput")
    # w[chunk, k_lo, (pair_in_chunk)*512 + kc*256 + o]: two 768 KiB slabs.
    w = nc.dram_tensor("w", [2, 128, PH * 2 * P2], MM_DT,
                       kind="ExternalInput")
    bias = nc.dram_tensor("bias", [128, PPC * 2], F32, kind="ExternalInput")
    # out[pair, oc, o_lo, t]
    out = nc.dram_tensor("out", [PPC, 2, 128, T], OUT_DT,
                         kind="ExternalOutput")

    with tile.TileContext(nc) as tc:
        with tc.tile_pool(name="biasp", bufs=1) as bias_pool, \
             tc.tile_pool(name="wp", bufs=1) as w_pool, \
             tc.tile_pool(name="xtp", bufs=6) as xt_pool, \
             tc.tile_pool(name="outp", bufs=4) as out_pool, \
             tc.tile_pool(name="psp", bufs=2, space="PSUM") as ps_pool:
            bias_sb = bias_pool.tile([128, PPC * 2], F32)
            nc.sync.dma_start(bias_sb[:], bias[:, :])
            # weights on the ACT ring (idle at start; stores come later)
            w_a = w_pool.tile([128, PH * 2 * P2], MM_DT)
            nc.scalar.dma_start(w_a[:], w[0])
            w_b = w_pool.tile([128, PH * 2 * P2], MM_DT)
            nc.scalar.dma_start(w_b[:], w[1])
            for pr in range(PPC):
                w_sb = w_a if pr < PH else w_b
                wb = (pr % PH) * 2 * P2
                xt_t = xt_pool.tile([128, 2 * T], MM_DT)
                nc.sync.dma_start(xt_t[:], xt[pr])
                for oc in range(2):
                    ps = ps_pool.tile([128, NB * 512], F32)
                    for nb in range(NB):
                        for kc in range(2):
                            nc.tensor.matmul(
                                ps[:, nb * 512:(nb + 1) * 512],
                                w_sb[:, wb + kc * P2 + oc * 128:
                                     wb + kc * P2 + oc * 128 + 128],
                                xt_t[:, kc * T + nb * 512:
                                     kc * T + (nb + 1) * 512],
                                start=(kc == 0), stop=(kc == 1))
                    out_t = out_pool.tile([128, T], OUT_DT)
                    bidx = pr * 2 + oc
                    nc.scalar.activation(
                        out_t[:], ps[:],
                        mybir.ActivationFunctionType.Tanh,
                        bias=bias_sb[:, bidx: bidx + 1],
                        scale=1.0)
                    nc.scalar.dma_start(out[pr, oc], out_t[:])
    nc.compile()
    return nc


def _pack_inputs(x, w_full, bias_full):
    # x (B, C, 512, 512) fp32 -> per-core xt[pair, k_lo, kc*2048 + t] slabs
    xp = x.astype(NP_MM).reshape(B, C, NH, PS, NW, PS)  # b c r py cl px
    sel = xp[:, :, _r, :, COLS, :]                      # g r b c py px
    xt = sel.transpose(3, 0, 4, 5, 2, 1).reshape(NPAIR, P2, T)
    xt = xt.reshape(NPAIR, 2, 128, T)
    xts = []
    for m in range(NCORES):
        sl = xt[m * PPC:(m + 1) * PPC].transpose(0, 2, 1, 3)
        xts.append(np.ascontiguousarray(sl.reshape(PPC, 128, 2 * T)))

    # w [c, g, p_in, o] -> per-core [chunk, k_lo, pair*512 + kc*256 + o]
    w2 = w_full.astype(NP_MM).reshape(NPAIR, 2, 128, P2)
    ws = []
    for m in range(NCORES):
        sl = w2[m * PPC:(m + 1) * PPC].reshape(2, PH, 2, 128, P2)
        ws.append(np.ascontiguousarray(
            sl.transpose(0, 3, 1, 2, 4).reshape(2, 128, PH * 2 * P2)))

    # bias [c, g, o] -> [o_lo, pair*2 + oc]
    b2 = bias_full.reshape(NPAIR, 2, 128)
    bs = []
    for m in range(NCORES):
        sl = b2[m * PPC:(m + 1) * PPC].transpose(2, 0, 1)
        bs.append(np.ascontiguousarray(sl.reshape(128, PPC * 2)))
    return xts, ws, bs


def _unpack_out(results, perm):
    # results[m]["out"]: [12, 2(oc), 128(o_lo), 2048(t)] fp16
    od = np.concatenate([results[m]["out"] for m in range(NCORES)])
    # [96, 2, 128, 2048] -> (c, g, oc, py_lo, px, b, r)
    od = od.reshape(C, G, 2, 8, PS, B, NH)
    src = od.transpose(1, 6, 5, 0, 2, 3, 4)            # g r b c oc py_lo px
    src = src.reshape(G, NH, B, C, PS, PS)             # py = oc*8 + py_lo
    tmp = np.empty((NH, NW, B, C, PS, PS), dtype=NP_MM)
    tmp[_r, COLS] = src                                # tmp[r, (g-r)%32]
    img = tmp.transpose(2, 3, 0, 4, 1, 5).reshape(B, C, IMG, IMG)
    return img[:, perm].astype(np.float32)


def kernel(x, obfuscation_weights, obfuscation_biases, channel_permutation):
    x = np.ascontiguousarray(x, dtype=np.float32)
    w = np.ascontiguousarray(obfuscation_weights, dtype=np.float32)
    bias = np.asarray(obfuscation_biases, dtype=np.float32)
    perm = np.asarray(channel_permutation, dtype=np.int64)

    if "nc" not in _CACHE:
        _CACHE["nc"] = _build_nc()
    nc = _CACHE["nc"]

    xts, ws, bs = _pack_inputs(x, w, bias)
    in_maps = [{"xt": xts[m], "w": ws[m], "bias": bs[m]}
               for m in range(NCORES)]

    res = run_bass_kernel_spmd(nc, in_maps, core_ids=list(range(NCORES)))
    _CACHE["last_results"] = res

    return _unpack_out(res.results, perm)


# revision 3
# speedup vs baseline: 1.5170x; 1.1256x over previous
"""Trainium2 Bass kernel for nn_ChannelWisePatchLevelObfuscator.

Math: split each (512,512) image into 32x32 patches of 16x16; per (channel,
group) apply a dense 256->256 obfuscation matmul over patch pixels (group =
(row+col) % 32), add bias, tanh, then permute channels.

Sharding: over the 96 (channel, group) pairs -- 12 pairs per core, each pair
covering the FULL batch (64 images x 32 patches = 2048 matmul rows). Unlike
batch-parallel sharding (which replicates the 12 MiB fp16 weight tensor into
every core), this loads each weight exactly once somewhere: per-core DMA
traffic drops from 36 MiB to 25.5 MiB (12 x + 1.5 w + 12 out), moving the
HBM roofline from ~106 us to ~75 us at 358 GB/s/core. The channel
permutation is applied for free while scattering per-core results into the
full output.

Schedule (the part that matters beyond traffic): the three DMA flows ride
three different issuing engines/queues so they never head-of-line block each
other and no compute engine pays descriptor-generation time it cannot
afford. All 12 x-slabs are dispatched up front on the SP ring (all tiles
resident, no pool-reuse throttling) so loads stream at full rate and finish
early; weights (4 small chunks, so the first matmul is gated by 384 KiB not
1.5 MiB) and bias ride the ACT ring which is otherwise idle; stores are
dispatched by the idle GPSIMD engine (SWDGE queue), leaving the scalar
engine to do nothing but its 24 big ACTIVATEs. The endgame is then a pure
store-drain at full bandwidth that hides the last pair's compute latency.

Precision: matmul inputs and the tanh output are fp16 (accumulation is fp32
in PSUM; bias+tanh on ScalarE reading fp32 PSUM). Rel err vs the fp32
reference ~3.6e-4.

Per pair: per output half oc, four N=512 matmul-pairs (K=2x128) accumulate
into a 4-bank [128,2048] PSUM tile, then one ScalarE activation does bias +
tanh + PSUM->SBUF fp16 over all 2048 columns (one big ACTIVATE amortizes the
~352-cycle fixed cost that made N=256 activations a 97us ScalarE
bottleneck); one 1 MiB SWDGE store per pair streams the result out.
"""
import sys
import numpy as np

sys.path.insert(0, "/opt/trn_rl_repo")

import concourse.bacc as bacc  # noqa: E402
import concourse.mybir as mybir  # noqa: E402
import concourse.tile as tile  # noqa: E402
from concourse.bass_utils import run_bass_kernel_spmd  # noqa: E402

IMG, C, PS, G, B = 512, 3, 16, 32, 64
NH = NW = IMG // PS          # 32 patches per side
P2 = PS * PS                 # 256 pixels per patch
NCORES = 8
NPAIR = C * G                # 96 (channel, group) pairs
PPC = NPAIR // NCORES        # 12 pairs per core
T = B * NH                   # 2048 matmul rows per pair: t = b*32 + r
NB = 4                       # N-blocks of 512 per oc half
NWC = 4                      # weight chunks per core
PWC = PPC // NWC             # pairs per weight chunk

F32 = mybir.dt.float32
MM_DT = mybir.dt.float16     # matmul input dtype
OUT_DT = mybir.dt.float16    # device store dtype; host upcasts to fp32
NP_MM = np.float16

_g = np.arange(G)[:, None]
_r = np.arange(NH)[None, :]
COLS = (_g - _r) % NW        # (g, r) -> patch column belonging to group g

_CACHE = {}


def _build_nc():
    nc = bacc.Bacc("TRN2", target_bir_lowering=False, debug=False,
                   num_devices=NCORES)
    # xt[pair, k_lo, kc*2048 + t]: contraction p = kc*128 + k_lo on
    # partitions; each pair is one contiguous 1 MiB slab (8 KiB/partition).
    xt = nc.dram_tensor("xt", [PPC, 128, 2 * T], MM_DT, kind="ExternalInput")
    # w[chunk, k_lo, (pair_in_chunk)*512 + kc*256 + o]: 384 KiB slabs.
    w = nc.dram_tensor("w", [NWC, 128, PWC * 2 * P2], MM_DT,
                       kind="ExternalInput")
    bias = nc.dram_tensor("bias", [128, PPC * 2], F32, kind="ExternalInput")
    # out[pair, o_lo, oc*2048 + t]
    out = nc.dram_tensor("out", [PPC, 128, 2 * T], OUT_DT,
                         kind="ExternalOutput")

    with tile.TileContext(nc) as tc:
        with tc.tile_pool(name="biasp", bufs=1) as bias_pool, \
             tc.tile_pool(name="wp", bufs=NWC) as w_pool, \
             tc.tile_pool(name="xtp", bufs=PPC) as xt_pool, \
             tc.tile_pool(name="outp", bufs=10) as out_pool, \
             tc.tile_pool(name="psp", bufs=2, space="PSUM") as ps_pool:
            # weights + bias on the ACT ring (otherwise idle)
            w_ts = []
            for ch in range(NWC):
                w_t = w_pool.tile([128, PWC * 2 * P2], MM_DT)
                nc.scalar.dma_start(w_t[:], w[ch])
                w_ts.append(w_t)
            bias_sb = bias_pool.tile([128, PPC * 2], F32)
            nc.scalar.dma_start(bias_sb[:], bias[:, :])
            # all x loads up front on the SP ring: every tile resident, so
            # load streaming is never throttled by pool reuse
            xt_ts = []
            for pr in range(PPC):
                xt_t = xt_pool.tile([128, 2 * T], MM_DT)
                nc.sync.dma_start(xt_t[:], xt[pr])
                xt_ts.append(xt_t)
            for pr in range(PPC):
                w_sb = w_ts[pr // PWC]
                wb = (pr % PWC) * 2 * P2
                xt_t = xt_ts[pr]
                out_t = out_pool.tile([128, 2 * T], OUT_DT)
                for oc in range(2):
                    ps = ps_pool.tile([128, NB * 512], F32)
                    for nb in range(NB):
                        for kc in range(2):
                            nc.tensor.matmul(
                                ps[:, nb * 512:(nb + 1) * 512],
                                w_sb[:, wb + kc * P2 + oc * 128:
                                     wb + kc * P2 + oc * 128 + 128],
                                xt_t[:, kc * T + nb * 512:
                                     kc * T + (nb + 1) * 512],
                                start=(kc == 0), stop=(kc == 1))
                    bidx = pr * 2 + oc
                    nc.scalar.activation(
                        out_t[:, oc * T:(oc + 1) * T], ps[:],
                        mybir.ActivationFunctionType.Tanh,
                        bias=bias_sb[:, bidx: bidx + 1],
                        scale=1.0)
                # store from the idle GPSIMD engine (SWDGE queue): keeps
                # descriptor generation off the scalar engine and off the
                # load ring
                nc.gpsimd.dma_start(out[pr], out_t[:])
    nc.compile()
    return nc


def _pack_inputs(x, w_full, bias_full):
    # x (B, C, 512, 512) fp32 -> per-core xt[pair, k_lo, kc*2048 + t] slabs
    xp = x.astype(NP_MM).reshape(B, C, NH, PS, NW, PS)  # b c r py cl px
    sel = xp[:, :, _r, :, COLS, :]                      # g r b c py px
    xt = sel.transpose(3, 0, 4, 5, 2, 1).reshape(NPAIR, P2, T)
    xt = xt.reshape(NPAIR, 2, 128, T)
    xts = []
    for m in range(NCORES):
        sl = xt[m * PPC:(m + 1) * PPC].transpose(0, 2, 1, 3)
        xts.append(np.ascontiguousarray(sl.reshape(PPC, 128, 2 * T)))

    # w [c, g, p_in, o] -> per-core [chunk, k_lo, pair*512 + kc*256 + o]
    w2 = w_full.astype(NP_MM).reshape(NPAIR, 2, 128, P2)
    ws = []
    for m in range(NCORES):
        sl = w2[m * PPC:(m + 1) * PPC].reshape(NWC, PWC, 2, 128, P2)
        ws.append(np.ascontiguousarray(
            sl.transpose(0, 3, 1, 2, 4).reshape(NWC, 128, PWC * 2 * P2)))

    # bias [c, g, o] -> [o_lo, pair*2 + oc]
    b2 = bias_full.reshape(NPAIR, 2, 128)
    bs = []
    for m in range(NCORES):
        sl = b2[m * PPC:(m + 1) * PPC].transpose(2, 0, 1)
        bs.append(np.ascontiguousarray(sl.reshape(128, PPC * 2)))
    return xts, ws, bs


def _unpack_out(results, perm):
    # results[m]["out"]: [12, 128(o_lo), 4096(oc*2048 + b*32 + r)] fp16
    od = np.concatenate([results[m]["out"] for m in range(NCORES)])
    od = od.reshape(C, G, 8, PS, 2, B, NH)             # c g py_lo px oc b r
    src = od.transpose(1, 6, 5, 0, 4, 2, 3)            # g r b c oc py_lo px
    src = src.reshape(G, NH, B, C, PS, PS)             # py = oc*8 + py_lo
    tmp = np.empty((NH, NW, B, C, PS, PS), dtype=NP_MM)
    tmp[_r, COLS] = src                                # tmp[r, (g-r)%32]
    img = tmp.transpose(2, 3, 0, 4, 1, 5).reshape(B, C, IMG, IMG)
    return img[:, perm].astype(np.float32)


def kernel(x, obfuscation_weights, obfuscation_biases, channel_permutation):
    x = np.ascontiguousarray(x, dtype=np.float32)
    w = np.ascontiguousarray(obfuscation_weights, dtype=np.float32)
    bias = np.asarray(obfuscation_biases, dtype=np.float32)
    perm = np.asarray(channel_permutation, dtype=np.int64)

    if "nc" not in _CACHE:
        _CACHE["nc"] = _build_nc()
    nc = _CACHE["nc"]

    xts, ws, bs = _pack_inputs(x, w, bias)
    in_maps = [{"xt": xts[m], "w": ws[m], "bias": bs[m]}
               for m in range(NCORES)]

    res = run_bass_kernel_spmd(nc, in_maps, core_ids=list(range(NCORES)))
    _CACHE["last_results"] = res

    return _unpack_out(res.results, perm)
